# revision 13
# baseline (speedup 1.0000x reference)
"""CapsuleNetwork Trainium2 kernel (Bass/Tile), 8-core data parallel.

Math reformulation (validated vs reference in fp32, rel err ~2e-6):
  primary p = x @ Wp + bp, viewed [B, n=8, d=16]
  squash scales gp[b,n] = gamma(||p_n||^2),  gamma(q) = q/(1+q)/sqrt(q+1e-8)
  u_hat_n = gp_n * (p_n @ W_n)
  Routing only needs the per-sample Gram matrix
      G[b,n,m] = u_hat_n . u_hat_m = gp_n gp_m * (p_n K_nm p_m),
  with K = Wflat @ Wflat^T precomputed on host ([128,128]).
  Routing loop runs on [B,8]/[B,8,8] tensors; final
      v = sum_m w_m u_hat_m = (p .* w_bcast) @ Wflat,  w = gam_last*c_last*gp.

Per-core layout: batch 2048 rows = 16 tiles of 128.  Stage 1 (per tile):
PE transposes x, computes primary^T, Z=K@p blocks, then mask-matmuls reduce
to sq[b,n] and Graw[b,n,m] directly in batch-major layout.  Routing runs
once on the wide [128, 16*...] accumulated tiles.  Stage 2 (per tile)
forms pw^T = p^T .* broadcast(w) and one K=128 matmul against Wflat.
"""

import numpy as np

import concourse.bass as bass
import concourse.tile as tile
from concourse import mybir
from concourse.bass_utils import run_bass_kernel_spmd
from concourse.vector_clock import ScopedClock

F32 = mybir.dt.float32
AF = mybir.ActivationFunctionType
AX = mybir.AxisListType

N_CORES = 8
B_FULL, IN_DIM, OUT_DIM = 16384, 1024, 512
N_CAPS, CAP_DIM = 8, 16
ND = N_CAPS * CAP_DIM          # 128
B_CORE = B_FULL // N_CORES     # 2048
P = 128                        # partitions / tile rows
K_CHUNKS = IN_DIM // P         # 8


def _patched_drain_and_barrier(self, tick_clock, wait_clock):
    # Walrus in this env allows at most ONE sem wait per instruction; the
    # stock tail drain accumulates one wait per live sem.  Collect waits on
    # a NOP, then re-emit one wait per NOP.
    nc = self.nc
    collector = nc.sync.nop()
    wait_clock.add_sem_waits(
        collector.ins, ScopedClock({None: tick_clock.global_clock})
    )
    si = collector.ins.sync_info
    waits = list(si.on_wait) if si is not None else []
    if len(waits) > 1:
        collector.ins.sync_info = mybir.SyncInfo(on_wait=waits[:1], on_update=[])
        for w in waits[1:]:
            n = nc.sync.nop()
            n.ins.sync_info = mybir.SyncInfo(on_wait=[w], on_update=[])
    nc.sync.drain()
    nc.all_engine_barrier()
    popped = nc._tile_sem_poison_stack.pop()
    assert popped is self._sem_poison
    nc.clear_and_free_semaphores(list(self.sems.allocated().values()))
    nc.all_engine_barrier()


tile.TileContext._drain_and_barrier = _patched_drain_and_barrier


def _split_multi_waits(nc):
    """Walrus here accepts at most one sem wait per instruction.  Tile's
    wait-assignment can attach several; split the extras onto single-wait
    NOPs inserted just before the instruction on the same engine."""
    k = 0
    for fn in nc.m.functions:
        for blk in fn.blocks:
            out = []
            for inst in blk.instructions:
                si = inst.sync_info
                if si is not None and len(si.on_wait) > 1:
                    waits = list(si.on_wait)
                    for w in waits[:-1]:
                        nop = mybir.InstNoOp(name=f"wsplit-{k}", ins=[], outs=[])
                        k += 1
                        nop.engine = inst.engine
                        nop.sync_info = mybir.SyncInfo(on_wait=[w], on_update=[])
                        nc.register_instruction(nop, overwrite=True)
                        out.append(nop)
                    inst.sync_info = mybir.SyncInfo(
                        on_wait=[waits[-1]], on_update=list(si.on_update)
                    )
                out.append(inst)
            blk.instructions = out


def build_nc(n_routing: int, n_tiles: int = B_CORE // P):
    nc = bass.Bass()
    rows = n_tiles * P

    x_ext = nc.declare_dram_parameter("x", [rows, IN_DIM], F32, isOutput=False)
    wpc_ext = nc.declare_dram_parameter("wpc", [P, IN_DIM], F32, isOutput=False)
    bp_ext = nc.declare_dram_parameter("bp", [ND, 1], F32, isOutput=False)
    wflat_ext = nc.declare_dram_parameter("wflat", [ND, OUT_DIM], F32, isOutput=False)
    kt_ext = nc.declare_dram_parameter("kt", [ND, N_CAPS * ND], F32, isOutput=False)
    nmask_ext = nc.declare_dram_parameter("nmask", [ND, N_CAPS], F32, isOutput=False)
    mmask_ext = nc.declare_dram_parameter("mmask", [N_CAPS, ND], F32, isOutput=False)
    ident_ext = nc.declare_dram_parameter("ident", [P, P], F32, isOutput=False)
    v_ext = nc.declare_dram_parameter("v", [rows, OUT_DIM], F32, isOutput=True)

    T = n_tiles

    with tile.TileContext(nc) as tc:
        with (
            tc.tile_pool(name="consts", bufs=1) as cpool,
            tc.tile_pool(name="persist", bufs=1) as ppool,
        ):
            wpc = cpool.tile([P, IN_DIM], F32)          # [p, k*128+j] = Wp[k*128+p, j]
            nc.sync.dma_start(wpc[:], wpc_ext[:])
            bp_sb = cpool.tile([ND, 1], F32)
            nc.sync.dma_start(bp_sb[:], bp_ext[:])
            wflat = cpool.tile([ND, OUT_DIM], F32)
            nc.sync.dma_start(wflat[:], wflat_ext[:])
            kt_sb = cpool.tile([ND, N_CAPS * ND], F32)  # [(n'd'), m*128+nd] masked
            nc.sync.dma_start(kt_sb[:], kt_ext[:])
            nmask = cpool.tile([ND, N_CAPS], F32)
            nc.sync.dma_start(nmask[:], nmask_ext[:])
            mmask = cpool.tile([N_CAPS, ND], F32)
            nc.sync.dma_start(mmask[:], mmask_ext[:])
            ident = cpool.tile([P, P], F32)
            nc.sync.dma_start(ident[:], ident_ext[:])
            eps_sb = cpool.tile([P, 1], F32)
            nc.gpsimd.memset(eps_sb[:], 1e-8)

            # Wide accumulators (whole-core)
            p_all = ppool.tile([P, T * ND], F32)        # primary^T per tile, [(n d), t*128 + b]? no: [:, t] slice is [nd, b]
            sq_all = ppool.tile([P, T * N_CAPS], F32)   # [b, t*8 + n]
            g_all = ppool.tile([P, T * 64], F32)        # [b, t*64 + m*8 + n]
            w_all = ppool.tile([P, T * N_CAPS], F32)    # [b, t*8 + m] final weights

            # ---------------- Stage 1: per-tile ----------------
            with (
                tc.tile_pool(name="xin", bufs=3) as xpool,
                tc.tile_pool(name="xt_ps", bufs=3, space="PSUM") as xt_ps_pool,
                tc.tile_pool(name="p_ps", bufs=1, space="PSUM") as p_ps_pool,
                tc.tile_pool(name="z_ps", bufs=2, space="PSUM") as z_ps_pool,
                tc.tile_pool(name="sg_ps", bufs=2, space="PSUM") as sg_ps_pool,
                tc.tile_pool(name="s1sb", bufs=2) as s1pool,
            ):
                for t in range(T):
                    x_sb = xpool.tile([P, IN_DIM], F32, tag="x")
                    nc.sync.dma_start(x_sb[:], x_ext[t * P:(t + 1) * P, :])

                    xt_sb = s1pool.tile([P, IN_DIM], F32, tag="xt")
                    for k in range(K_CHUNKS):
                        xt_ps = xt_ps_pool.tile([P, P], F32, tag="xtps")
                        nc.tensor.transpose(
                            xt_ps[:], x_sb[:, k * P:(k + 1) * P], ident[:]
                        )
                        eng = nc.scalar if (k % 2 == 0) else nc.vector
                        if eng is nc.scalar:
                            eng.copy(xt_sb[:, k * P:(k + 1) * P], xt_ps[:])
                        else:
                            eng.tensor_copy(xt_sb[:, k * P:(k + 1) * P], xt_ps[:])

                    p_ps = p_ps_pool.tile([P, P], F32, tag="pps")
                    for k in range(K_CHUNKS):
                        nc.tensor.matmul(
                            p_ps[:],
                            wpc[:, k * P:(k + 1) * P],
                            xt_sb[:, k * P:(k + 1) * P],
                            start=(k == 0),
                            stop=(k == K_CHUNKS - 1),
                        )
                    # primary^T with bias -> persistent SBUF
                    p_sb = p_all[:, t * ND:(t + 1) * ND]
                    nc.scalar.activation(
                        p_sb, p_ps[:], AF.Identity, bias=bp_sb[:, 0:1], scale=1.0
                    )

                    # squared primary^T
                    p2_sb = s1pool.tile([P, P], F32, tag="p2")
                    nc.scalar.activation(p2_sb[:], p_sb, AF.Square)

                    # sq[b, n] = sum_d p^2 : mask matmul, p2 stationary.
                    # sq and G share one PSUM tile (bank budget).
                    sg_ps = sg_ps_pool.tile([P, 72], F32, tag="sg")
                    sq_ps = sg_ps[:, 64:72]
                    g_ps = sg_ps[:, 0:64]
                    nc.tensor.matmul(sq_ps, p2_sb[:], nmask[:], start=True, stop=True)
                    nc.scalar.copy(sq_all[:, t * N_CAPS:(t + 1) * N_CAPS], sq_ps)

                    # G raw: Z blocks then E = Z .* p_bcast then mask matmuls
                    for h in range(2):
                        z_ps = z_ps_pool.tile([P, 4 * P], F32, tag="zps")
                        for mm in range(4):
                            m = 4 * h + mm
                            nc.tensor.matmul(
                                z_ps[:, mm * P:(mm + 1) * P],
                                kt_sb[:, m * ND:(m + 1) * ND],
                                p_sb,
                                start=True,
                                stop=True,
                            )
                        e_sb = s1pool.tile([P, 4 * P], F32, tag="esb")
                        p_bcast = (
                            p_sb.rearrange("p (o b) -> p o b", o=1)
                            .to_broadcast((P, 4, P))
                        )
                        nc.vector.tensor_mul(
                            e_sb[:].rearrange("p (m b) -> p m b", m=4),
                            z_ps[:].rearrange("p (m b) -> p m b", m=4),
                            p_bcast,
                        )
                        for mm in range(4):
                            m = 4 * h + mm
                            nc.tensor.matmul(
                                g_ps[:, m * 8:(m + 1) * 8],
                                e_sb[:, mm * P:(mm + 1) * P],
                                nmask[:],
                                start=True,
                                stop=True,
                            )
                    nc.scalar.copy(g_all[:, t * 64:(t + 1) * 64], g_ps[:])

            # ---------------- Routing (batched over all tiles) ----------------
            with tc.tile_pool(name="rt", bufs=1) as rt:
                TN = T * N_CAPS            # 128-wide per-(t,n)
                T64 = T * 64

                def bcast_tn(src, over):  # src [P, T, 8] -> [P, T, 8, 8]
                    if over == "n":  # broadcast over trailing n (src idx = m)
                        ap = src.rearrange("p (t c o) -> p t c o", t=T, o=1)
                        return ap.to_broadcast((P, T, 8, 8))
                    else:  # over m: src idx = n
                        ap = src.rearrange("p (t o c) -> p t o c", t=T, o=1)
                        return ap.to_broadcast((P, T, 8, 8))

                def gamma_of(q, width):  # q [P, width] -> gamma(q), 5 ops
                    sq1 = rt.tile([P, width], F32, tag=f"ga{width}")
                    nc.scalar.activation(sq1[:], q, AF.Sqrt, bias=eps_sb[:, 0:1])
                    qp1 = rt.tile([P, width], F32, tag=f"gb{width}")
                    nc.vector.tensor_scalar_add(qp1[:], q, 1.0)
                    den = rt.tile([P, width], F32, tag=f"gc{width}")
                    nc.vector.tensor_mul(den[:], sq1[:], qp1[:])
                    rden = rt.tile([P, width], F32, tag=f"gd{width}")
                    nc.vector.reciprocal(rden[:], den[:])
                    gam = rt.tile([P, width], F32, tag=f"ge{width}")
                    nc.vector.tensor_mul(gam[:], q, rden[:])
                    return gam

                gp = gamma_of(sq_all[:], TN)           # [P, (t,n)]

                # G *= gp_m ; G *= gp_n     (layout [t, m, n])
                g_v = g_all[:].rearrange("p (t m n) -> p t m n", t=T, m=8)
                nc.vector.tensor_mul(g_v, g_v, bcast_tn(gp[:], "n"))
                nc.vector.tensor_mul(g_v, g_v, bcast_tn(gp[:], "m"))

                blog = rt.tile([P, TN], F32, tag="blog")
                c_t = rt.tile([P, TN], F32, tag="ct")
                gam = None
                c_uniform = True
                for i in range(n_routing):
                    if i > 0:
                        # softmax over n of blog
                        e_t = rt.tile([P, TN], F32, tag="et")
                        nc.scalar.activation(e_t[:], blog[:], AF.Exp)
                        ssum = rt.tile([P, T], F32, tag="ssum")
                        nc.vector.reduce_sum(
                            ssum[:],
                            e_t[:].rearrange("p (t n) -> p t n", t=T),
                            axis=AX.X,
                        )
                        rsum = rt.tile([P, T], F32, tag="rsum")
                        nc.vector.reciprocal(rsum[:], ssum[:])
                        r_b = (
                            rsum[:].rearrange("p (t o) -> p t o", o=1)
                            .to_broadcast((P, T, 8))
                        )
                        nc.vector.tensor_mul(
                            c_t[:].rearrange("p (t n) -> p t n", t=T),
                            e_t[:].rearrange("p (t n) -> p t n", t=T),
                            r_b,
                        )
                        c_uniform = False

                    # g = G @ c   (sum over m);  layout trick: view [t, n, m]
                    g_i = rt.tile([P, TN], F32, tag="gi")
                    if c_uniform:
                        nc.vector.reduce_sum(
                            g_i[:].rearrange("p (t n) -> p t n", t=T),
                            g_all[:].rearrange("p (t m n) -> p t n m", t=T, m=8),
                            axis=AX.X,
                        )
                        nc.vector.tensor_scalar_mul(g_i[:], g_i[:], 1.0 / N_CAPS)
                    else:
                        gc = rt.tile([P, T64], F32, tag="gc_big")
                        nc.vector.tensor_mul(
                            gc[:].rearrange("p (t m n) -> p t m n", t=T, m=8),
                            g_v,
                            bcast_tn(c_t[:], "n"),
                        )
                        nc.vector.reduce_sum(
                            g_i[:].rearrange("p (t n) -> p t n", t=T),
                            gc[:].rearrange("p (t m n) -> p t n m", t=T, m=8),
                            axis=AX.X,
                        )

                    # q = c . g
                    q_t = rt.tile([P, T], F32, tag="qt")
                    if c_uniform:
                        nc.vector.reduce_sum(
                            q_t[:],
                            g_i[:].rearrange("p (t n) -> p t n", t=T),
                            axis=AX.X,
                        )
                        nc.vector.tensor_scalar_mul(q_t[:], q_t[:], 1.0 / N_CAPS)
                    else:
                        cg = rt.tile([P, TN], F32, tag="cg")
                        nc.vector.tensor_mul(cg[:], c_t[:], g_i[:])
                        nc.vector.reduce_sum(
                            q_t[:],
                            cg[:].rearrange("p (t n) -> p t n", t=T),
                            axis=AX.X,
                        )

                    gam = gamma_of(q_t[:], T)          # [P, T]

                    gam_b = (
                        gam[:].rearrange("p (t o) -> p t o", o=1)
                        .to_broadcast((P, T, 8))
                    )
                    if i < n_routing - 1:
                        gg = rt.tile([P, TN], F32, tag="gg")
                        nc.vector.tensor_mul(
                            gg[:].rearrange("p (t n) -> p t n", t=T),
                            g_i[:].rearrange("p (t n) -> p t n", t=T),
                            gam_b,
                        )
                        if i == 0:
                            nc.vector.tensor_copy(blog[:], gg[:])
                        else:
                            nc.vector.tensor_add(blog[:], blog[:], gg[:])
                    else:
                        # final weights w = gam * c * gp   (c uniform if n_routing==1)
                        if c_uniform:
                            nc.vector.tensor_scalar_mul(c_t[:], gp[:], 1.0 / N_CAPS)
                            src = c_t
                        else:
                            nc.vector.tensor_mul(c_t[:], c_t[:], gp[:])
                            src = c_t
                        nc.vector.tensor_mul(
                            w_all[:].rearrange("p (t n) -> p t n", t=T),
                            src[:].rearrange("p (t n) -> p t n", t=T),
                            gam_b,
                        )

            # ---------------- Stage 2: per-tile output ----------------
            with (
                tc.tile_pool(name="s2ps", bufs=2, space="PSUM") as s2ps,
                tc.tile_pool(name="v_ps", bufs=2, space="PSUM") as vps,
                tc.tile_pool(name="s2sb", bufs=2) as s2sb,
            ):
                for t in range(T):
                    wt_ps = s2ps.tile([N_CAPS, P], F32, tag="wtps")
                    nc.tensor.transpose(
                        wt_ps[:], w_all[:, t * N_CAPS:(t + 1) * N_CAPS], ident[:, 0:P]
                    )
                    wt_sb = s2sb.tile([N_CAPS, P], F32, tag="wtsb")
                    nc.scalar.copy(wt_sb[:], wt_ps[:])

                    wb_ps = s2ps.tile([P, P], F32, tag="wbps")
                    nc.tensor.matmul(wb_ps[:], mmask[:], wt_sb[:], start=True, stop=True)

                    pw_sb = s2sb.tile([P, P], F32, tag="pwsb")
                    nc.vector.tensor_mul(
                        pw_sb[:], p_all[:, t * ND:(t + 1) * ND], wb_ps[:]
                    )

                    v_ps = vps.tile([P, OUT_DIM], F32, tag="vps")
                    nc.tensor.matmul(v_ps[:], pw_sb[:], wflat[:], start=True, stop=True)

                    v_sb = s2sb.tile([P, OUT_DIM], F32, tag="vsb")
                    nc.scalar.copy(v_sb[:], v_ps[:])
                    nc.sync.dma_start(v_ext[t * P:(t + 1) * P, :], v_sb[:])

    _split_multi_waits(nc)
    return nc


def _host_consts(Wp, bp, W):
    Wp = np.asarray(Wp, dtype=np.float32)
    bp = np.asarray(bp, dtype=np.float32)
    W = np.asarray(W, dtype=np.float32)
    wflat = W.reshape(ND, OUT_DIM)
    Kmat = wflat @ wflat.T                                    # [128, 128]
    # wpc[p, k*128+j] = Wp[k*128+p, j]
    wpc = Wp.reshape(K_CHUNKS, P, P).transpose(1, 0, 2).reshape(P, IN_DIM)
    # kt[(n'd'), m*128+(nd)] = delta_{n' m} * K[nd, m*16+d']  (block-masked lhsT)
    kt = np.zeros((ND, N_CAPS * ND), dtype=np.float32)
    for m in range(N_CAPS):
        kt[m * CAP_DIM:(m + 1) * CAP_DIM, m * ND:(m + 1) * ND] = \
            Kmat[:, m * CAP_DIM:(m + 1) * CAP_DIM].T
    nmask = np.zeros((ND, N_CAPS), dtype=np.float32)
    for n in range(N_CAPS):
        nmask[n * CAP_DIM:(n + 1) * CAP_DIM, n] = 1.0
    mmask = np.zeros((N_CAPS, ND), dtype=np.float32)
    for m in range(N_CAPS):
        mmask[m, m * CAP_DIM:(m + 1) * CAP_DIM] = 1.0
    ident = np.eye(P, dtype=np.float32)
    return {
        "wpc": np.ascontiguousarray(wpc),
        "bp": np.ascontiguousarray(bp.reshape(ND, 1)),
        "wflat": np.ascontiguousarray(wflat),
        "kt": np.ascontiguousarray(kt),
        "nmask": nmask,
        "mmask": mmask,
        "ident": ident,
    }


_NC_CACHE = {}
TRACE = False          # test harness sets True to collect an NTFF profile
LAST_RESULT = None     # BassKernelResults of the most recent run


def kernel(x, Wp, bp, W, n_routing):
    n_routing = int(n_routing)
    x = np.ascontiguousarray(np.asarray(x, dtype=np.float32))
    assert x.shape == (B_FULL, IN_DIM)

    key = (n_routing,)
    if key not in _NC_CACHE:
        _NC_CACHE[key] = build_nc(n_routing)
    nc = _NC_CACHE[key]

    consts = _host_consts(Wp, bp, W)
    in_maps = []
    for c in range(N_CORES):
        m = {"x": x[c * B_CORE:(c + 1) * B_CORE, :]}
        m.update(consts)
        in_maps.append(m)

    global LAST_RESULT
    res = run_bass_kernel_spmd(nc, in_maps, list(range(N_CORES)), trace=TRACE)
    LAST_RESULT = res
    out = np.concatenate([res.results[c]["v"] for c in range(N_CORES)], axis=0)
    return out.astype(np.float32)
